# revision 20
# baseline (speedup 1.0000x reference)
"""Trainium2 Bass kernel for nn_Encoder (2-layer GCN encoder, graph mean readout).

Math restructuring (exact, up to float reordering):
  Layer 1 (GCNConv + ReLU), aggregate-then-transform (GCN linearity):
      a1[n] = dis[n] * (S[n] @ W1ext),  S[n] = sum_{e in seg(n)} g[src(e)]
      where the segment includes a self edge, g[m] = dis[m] * x_ext[m],
      dis = (deg+1)^-1/2, x_ext = [node feats | one-hot(node_type)].
      x1[n] = relu(a1[n] + b1).
  Layer 2 + mean over nodes collapses to a per-node scalar:
      out = (1/N) * (sum_n c[n] * x1[n]) @ W2 + b2,
      c[m] = dis[m] * (sum_{e: src(e)=m} dis[dst(e)] + dis[m]).

Device-side work per core (dst-sharded, 1/8 of nodes + their in-edges):
  1. Stream the fp8 edge message rows (g[src], host lays the rows out in
     exact consumption order, two edges packed per lane for DoubleRow) and
     the fp8 one-hot slot masks with large sequential DMAs, split over
     two DMA rails (gpsimd SWDGE + sync HWDGE ring) so many SDMA
     engines pull concurrently.  This sidesteps the two baseline
     bottlenecks: SWDGE dma_gather descriptor generation (measured
     8.4 ns/descriptor of serial GPSIMD -> 1.8 ms for 200k edges) and the
     4-engine striping of a single HWDGE queue.
  2. Segment-sum via one-hot matmuls into PSUM, feature-major, in fp8
     DoubleRow perf mode (two 128-edge chunks contracted per matmul):
     psum[f, slot] += sum_e gt[e, f] * oh[e, slot].
  3. z = a1T @ W1ext per batch (two fp16 matmuls, no transposes needed
     since the aggregation already produced feature-major layout), then
     x1c = relu(z * dis*c) per slot on ACT, accumulated into acc on DVE.
  4. Host sums acc over slots and cores, applies the tiny [2,128] @ W2.

Sharding: destination nodes (and the incoming-edge stream, partitioned by
destination) across 8 cores; weights replicated; SPMD single program.
"""

import sys, os, types
sys.path.insert(0, "/opt/trn_rl_repo")

# antenv.axon_hooks shim (image's antenv stub lacks it); needed for NTFF trace.
if "antenv.axon_hooks" not in sys.modules:
    _hook = [None]
    _m = types.ModuleType("antenv.axon_hooks")
    _m.set_axon_ntff_profile_hook = lambda h: _hook.__setitem__(0, h)
    _m.get_axon_ntff_profile_hook = lambda: _hook[0]
    sys.modules["antenv.axon_hooks"] = _m
    try:
        import antenv
        antenv.axon_hooks = _m
        from trn_agent_boot.trn_boot import _ntff_profile_via_ctypes
        _m.set_axon_ntff_profile_hook(
            _ntff_profile_via_ctypes("/opt/axon/libaxon_pjrt.so"))
    except Exception:
        pass

import numpy as np
import ml_dtypes
from dataclasses import dataclass

import concourse.bacc as bacc
import concourse.bass as bass
import concourse.mybir as mybir
import concourse.tile as tile
from concourse.bass_utils import run_bass_kernel_spmd

P = 128
H = 128
F_IN = 116
FEXT = F_IN + 8          # 124: features + one-hot(type) per batch
B = 2
ROW = 256                # g row: [b0 feats 124 | pad 4 | b1 feats 124 | pad 4]
PROW = 512               # paired stream row: [A_b0 | B_b0 | A_b1 | B_b1]
YW = B * H               # 256 output cols (both batches)

F8 = ml_dtypes.float8_e4m3


@dataclass(frozen=True)
class Cfg:
    n: int = 100000      # nodes
    ncores: int = 8
    tiles: int = 98      # dst tiles per core (128 slots each)
    pairs: int = 9       # chunk pairs per tile (2*128-edge chunks each)

    @property
    def chunks(self):
        return 2 * self.pairs

    @property
    def ndst(self):
        return -(-self.n // self.ncores)

    @property
    def npairs_total(self):
        return self.tiles * self.pairs

    @property
    def nchunks_total(self):
        return self.tiles * self.chunks


CFG = Cfg()

f32 = mybir.dt.float32
f16 = mybir.dt.float16
f8 = mybir.dt.float8e4


def _build_program(cfg: Cfg, has_b1: bool):
    nc = bacc.Bacc("TRN2")
    # pair dim kept explicit: [part, paircol, b-half, A/B, 128] — the
    # DoubleRow matmul requires lhsT APs shaped [K, 2, M]
    strm = nc.dram_tensor("strm", [P, cfg.npairs_total, 2, 2, P], f8,
                          kind="ExternalInput")
    oht = nc.dram_tensor("oht", [P, cfg.nchunks_total, P], f8,
                         kind="ExternalInput")
    dcq = nc.dram_tensor("dcq", [P, cfg.tiles], f32, kind="ExternalInput")
    w1p = nc.dram_tensor("w1p", [P, H], f16, kind="ExternalInput")
    if has_b1:
        disc = nc.dram_tensor("disc", [P, cfg.tiles], f32, kind="ExternalInput")
        cct = nc.dram_tensor("cct", [P, cfg.tiles], f32, kind="ExternalInput")
        b1b = nc.dram_tensor("b1b", [P, YW], f32, kind="ExternalInput")
    accd = nc.dram_tensor("acc", [P, YW], f32, kind="ExternalOutput")

    hc = cfg.pairs  # oh chunks on the scalar rail (first half); rest on sync

    with tile.TileContext(nc) as tc:
        with (
            tc.tile_pool(name="const", bufs=1) as cpool,
            tc.tile_pool(name="gt", bufs=4) as gtpool,
            tc.tile_pool(name="oh", bufs=4) as ohpool,
            tc.tile_pool(name="a1", bufs=6) as a1pool,
            tc.tile_pool(name="x1c", bufs=4) as xpool,
            tc.tile_pool(name="psa", bufs=3, space="PSUM") as psa,
            tc.tile_pool(name="psz", bufs=3, space="PSUM") as psz,
        ):
            w1_sb = cpool.tile([P, H], f16, tag="w1")
            nc.sync.dma_start(w1_sb[:], w1p[:])
            dcq_sb = cpool.tile([P, cfg.tiles], f32, tag="dcq")
            nc.sync.dma_start(dcq_sb[:], dcq[:])
            if has_b1:
                disc_sb = cpool.tile([P, cfg.tiles], f32, tag="disc")
                nc.sync.dma_start(disc_sb[:], disc[:])
                cc_sb = cpool.tile([P, cfg.tiles], f32, tag="cc")
                nc.sync.dma_start(cc_sb[:], cct[:])
                b1_sb = cpool.tile([P, YW], f32, tag="b1b")
                nc.sync.dma_start(b1_sb[:], b1b[:])
            acc_sb = cpool.tile([P, YW], f32, tag="acc")
            nc.vector.memset(acc_sb[:], 0)

            for t in range(cfg.tiles):
                p0 = t * cfg.pairs
                c0 = t * cfg.chunks
                gt = gtpool.tile([P, cfg.pairs, 2, 2, P], f8, tag="gt")
                nc.gpsimd.dma_start(gt[:], strm[:, p0:p0 + cfg.pairs])
                oh = ohpool.tile([P, cfg.chunks, P], f8, tag="oh")
                nc.gpsimd.dma_start(oh[:, 0:hc, :], oht[:, c0:c0 + hc, :])
                nc.sync.dma_start(oh[:, hc:cfg.chunks, :],
                                  oht[:, c0 + hc:c0 + cfg.chunks, :])

                # feature-major segment sum: ps[f, slot], fp8 DoubleRow
                # (each matmul contracts a pair of 128-edge chunks)
                ps = psa.tile([P, 512], f32, tag="psa")
                start_mm = None
                for jp in range(cfg.pairs):
                    for h in range(2):
                        mm = nc.tensor.matmul(
                            ps[:, h * P:(h + 1) * P],
                            lhsT=gt[:, jp, h],
                            rhs=oh[:, 2 * jp:2 * jp + 2, :],
                            start=(jp == 0 and h == 0),
                            stop=(jp == cfg.pairs - 1 and h == 1),
                            perf_mode=mybir.MatmulPerfMode.DoubleRow)
                        if start_mm is None:
                            start_mm = mm
                        else:
                            bass._add_dep_helper(
                                mm.ins, start_mm.ins, sync=False,
                                reason="accum after psum start")

                a01 = a1pool.tile([P, YW], f16, tag="a01")
                nc.vector.tensor_copy(a01[:], ps[:, 0:YW])

                pz = psz.tile([P, 512], f32, tag="psz")
                z0 = nc.tensor.matmul(pz[:, 0:P], lhsT=a01[:, 0:P],
                                      rhs=w1_sb[:], start=True, stop=False)
                z1 = nc.tensor.matmul(pz[:, P:2 * P], lhsT=a01[:, P:2 * P],
                                      rhs=w1_sb[:], start=False, stop=True)
                bass._add_dep_helper(z1.ins, z0.ins, sync=False,
                                     reason="z1 after psum start")

                x1c = xpool.tile([P, YW], f32, tag="x1c")
                if not has_b1:
                    # x1c = relu(z * (dis*c))   (valid since c>0)
                    nc.scalar.activation(
                        out=x1c[:], in_=pz[:, 0:YW],
                        func=mybir.ActivationFunctionType.Relu,
                        bias=0.0, scale=dcq_sb[:, t:t + 1])
                else:
                    t1 = xpool.tile([P, YW], f32, tag="t1")
                    nc.vector.tensor_scalar(
                        out=t1[:], in0=pz[:, 0:YW],
                        scalar1=disc_sb[:, t:t + 1], scalar2=None,
                        op0=mybir.AluOpType.mult)
                    nc.vector.tensor_tensor(
                        out=t1[:], in0=t1[:], in1=b1_sb[:],
                        op=mybir.AluOpType.add)
                    nc.scalar.activation(
                        out=t1[:], in_=t1[:],
                        func=mybir.ActivationFunctionType.Relu)
                    nc.vector.tensor_scalar(
                        out=x1c[:], in0=t1[:],
                        scalar1=cc_sb[:, t:t + 1], scalar2=None,
                        op0=mybir.AluOpType.mult)
                nc.vector.tensor_tensor(
                    out=acc_sb[:], in0=acc_sb[:], in1=x1c[:],
                    op=mybir.AluOpType.add)

            nc.sync.dma_start(accd[:], acc_sb[:])

    nc.compile()
    return nc


_PROG_CACHE = {}


def _get_program(cfg: Cfg, has_b1: bool):
    key = (cfg, has_b1)
    if key not in _PROG_CACHE:
        _PROG_CACHE[key] = _build_program(cfg, has_b1)
    return _PROG_CACHE[key]


def _pack_core(cfg: Cfg, cnt):
    """Bin-pack local dst nodes (cnt = edges incl. self per node) into tiles
    of <=128 slots and <=chunks*128 edges.  Returns (tile_of, slot_of) or
    None if infeasible."""
    ndst = len(cnt)
    cap_e = np.full(cfg.tiles, cfg.chunks * P, dtype=np.int64)
    cap_s = np.full(cfg.tiles, P, dtype=np.int64)
    tile_of = np.empty(ndst, dtype=np.int64)
    slot_of = np.empty(ndst, dtype=np.int64)
    order = np.argsort(-cnt, kind="stable")
    for nloc in order:
        need = cnt[nloc]
        score = np.where((cap_e >= need) & (cap_s > 0), cap_e, -1)
        t = int(np.argmax(score))
        if score[t] < 0:
            return None
        tile_of[nloc] = t
        slot_of[nloc] = P - cap_s[t]
        cap_s[t] -= 1
        cap_e[t] -= need
    return tile_of, slot_of


def _prepare(cfg: Cfg, node, node_type, edge_index, embed, W1, b1, W2, b2):
    n = cfg.n
    src = edge_index[0].astype(np.int64)
    dst = edge_index[1].astype(np.int64)
    deg = (np.bincount(dst, minlength=n) + 1).astype(np.float64)
    dis = 1.0 / np.sqrt(deg)
    s_arr = np.bincount(src, weights=dis[dst], minlength=n)
    c = dis * (s_arr + dis)
    dis_c = (dis * c).astype(np.float32)
    dis32 = dis.astype(np.float32)

    # message-row table: g[n] = dis[n] * [x_b0 | onehot | pad | x_b1 | onehot | pad]
    xg = np.zeros((n, ROW), dtype=F8)
    for b in range(B):
        o = b * 128
        xg[:, o:o + F_IN] = (node[b] * dis32[:, None]).astype(F8)
        oh = np.zeros((n, 8), dtype=np.float32)
        oh[np.arange(n), node_type.astype(np.int64)] = dis32
        xg[:, o + F_IN:o + FEXT] = oh.astype(F8)

    T8 = embed.astype(np.float64) @ W1[F_IN:, :].astype(np.float64)
    w1p = np.zeros((P, H), dtype=np.float16)
    w1p[:F_IN] = W1[:F_IN].astype(np.float16)
    w1p[F_IN:FEXT] = T8.astype(np.float16)

    eye8 = np.eye(P, dtype=F8)

    has_b1 = bool(np.any(b1 != 0))
    in_maps = []
    for core in range(cfg.ncores):
        n0 = core * cfg.ndst
        n1 = min(n0 + cfg.ndst, n)
        sel = (dst >= n0) & (dst < n1)
        es = src[sel]
        edl = dst[sel] - n0
        # append self edges
        own = np.arange(n0, n1, dtype=np.int64)
        es = np.concatenate([es, own])
        edl = np.concatenate([edl, own - n0])

        cnt = np.bincount(edl, minlength=n1 - n0)
        pack = _pack_core(cfg, cnt)
        if pack is None:
            raise RuntimeError(f"core {core}: bin packing failed "
                               f"(tiles={cfg.tiles}, chunks={cfg.chunks})")
        tile_of, slot_of = pack

        et = tile_of[edl]
        order = np.argsort(et, kind="stable")
        et_s = et[order]
        src_s = es[order]
        slot_s = slot_of[edl][order]
        starts = np.concatenate(
            [[0], np.cumsum(np.bincount(et_s, minlength=cfg.tiles))[:-1]])
        rank = np.arange(len(et_s)) - starts[et_s]
        chunk = rank // P
        lane = rank % P
        jp = chunk // 2        # pair index within tile
        ab = chunk % 2         # 0 = A (first of pair), 1 = B
        pc = et_s * cfg.pairs + jp

        strm = np.zeros((P, cfg.npairs_total, PROW), dtype=F8)
        rows = xg[src_s]                               # [E, 256]
        colbase = (ab * P)[:, None] + np.arange(P)[None, :]   # [E, 128]
        strm[lane[:, None], pc[:, None], colbase] = rows[:, 0:P]
        strm[lane[:, None], pc[:, None], 2 * P + colbase] = rows[:, P:2 * P]

        oht = np.zeros((P, cfg.nchunks_total, P), dtype=F8)
        oht[lane, et_s * cfg.chunks + chunk] = eye8[slot_s]

        dcq_w = np.zeros((P, cfg.tiles), dtype=np.float32)
        dcq_w[slot_of, tile_of] = dis_c[n0:n1]

        m = {"strm": strm.reshape(P, cfg.npairs_total, 2, 2, P),
             "oht": oht, "dcq": dcq_w, "w1p": w1p}
        if has_b1:
            disc_w = np.zeros((P, cfg.tiles), dtype=np.float32)
            cc_w = np.zeros((P, cfg.tiles), dtype=np.float32)
            disc_w[slot_of, tile_of] = dis32[n0:n1]
            cc_w[slot_of, tile_of] = c[n0:n1].astype(np.float32)
            m["disc"] = disc_w
            m["cct"] = cc_w
            m["b1b"] = np.tile(b1.astype(np.float32), (P, B))
        in_maps.append(m)
    return in_maps, has_b1


def run(inputs, cfg: Cfg = CFG, trace: bool = False, trace_cores=None):
    node = np.asarray(inputs["node"], dtype=np.float32)
    node_type = np.asarray(inputs["node_type"])
    edge_index = np.asarray(inputs["edge_index"])
    embed = np.asarray(inputs["embed"], dtype=np.float32)
    W1 = np.asarray(inputs["W1"], dtype=np.float32)
    b1 = np.asarray(inputs["b1"], dtype=np.float32)
    W2 = np.asarray(inputs["W2"], dtype=np.float32)
    b2 = np.asarray(inputs["b2"], dtype=np.float32)

    while True:
        try:
            in_maps, has_b1 = _prepare(cfg, node, node_type, edge_index,
                                       embed, W1, b1, W2, b2)
            break
        except RuntimeError:
            # packing infeasible for this edge distribution: add capacity
            cfg = Cfg(n=cfg.n, ncores=cfg.ncores, tiles=cfg.tiles + 2,
                      pairs=cfg.pairs)
    nc = _get_program(cfg, has_b1)
    if trace_cores is None:
        trace_cores = list(range(cfg.ncores))
    res = run_bass_kernel_spmd(
        nc, in_maps, core_ids=list(range(cfg.ncores)), trace=trace,
        trace_cores=trace_cores if trace else None)

    total = np.zeros((B, H), dtype=np.float64)
    for core in range(cfg.ncores):
        acc = res.results[core]["acc"].astype(np.float64)   # [128, 2*H]
        total += acc.reshape(P, B, H).sum(axis=0)
    out = (total @ W2.astype(np.float64)) / cfg.n + b2.astype(np.float64)
    return out.astype(np.float32), res


def kernel(**inputs) -> np.ndarray:
    out, _ = run(inputs, CFG, trace=False)
    return out


# revision 22
# speedup vs baseline: 1.2376x; 1.2376x over previous
"""Trainium2 Bass kernel for nn_Encoder (2-layer GCN encoder, graph mean readout).

Math restructuring (exact, up to float reordering):
  Layer 1 (GCNConv + ReLU), aggregate-then-transform (GCN linearity):
      a1[n] = dis[n] * (S[n] @ W1ext),  S[n] = sum_{e in seg(n)} g[src(e)]
      where the segment includes a self edge, g[m] = dis[m] * x_ext[m],
      dis = (deg+1)^-1/2, x_ext = [node feats | one-hot(node_type)].
      x1[n] = relu(a1[n] + b1).
  Layer 2 + mean over nodes collapses to a per-node scalar:
      out = (1/N) * (sum_n c[n] * x1[n]) @ W2 + b2,
      c[m] = dis[m] * (sum_{e: src(e)=m} dis[dst(e)] + dis[m]).

Device-side work per core (dst-sharded, 1/8 of nodes + their in-edges):
  1. Stream the fp8 edge message rows (g[src], host lays the rows out in
     exact consumption order, two edges packed per lane for DoubleRow) and
     the fp8 one-hot slot masks with large sequential DMAs, split over
     two DMA rails (gpsimd SWDGE + sync HWDGE ring) so many SDMA
     engines pull concurrently.  This sidesteps the two baseline
     bottlenecks: SWDGE dma_gather descriptor generation (measured
     8.4 ns/descriptor of serial GPSIMD -> 1.8 ms for 200k edges) and the
     4-engine striping of a single HWDGE queue.
  2. Segment-sum via one-hot matmuls into PSUM, feature-major, in fp8
     DoubleRow perf mode (two 128-edge chunks contracted per matmul):
     psum[f, slot] += sum_e gt[e, f] * oh[e, slot].
  3. z = a1T @ W1ext per batch (two fp16 matmuls, no transposes needed
     since the aggregation already produced feature-major layout), then
     x1c = relu(z * dis*c) per slot on ACT, accumulated into acc on DVE.
  4. Host sums acc over slots and cores, applies the tiny [2,128] @ W2.

Sharding: destination nodes (and the incoming-edge stream, partitioned by
destination) across 8 cores; weights replicated; SPMD single program.
"""

import sys, os, types
sys.path.insert(0, "/opt/trn_rl_repo")

# antenv.axon_hooks shim (image's antenv stub lacks it); needed for NTFF trace.
if "antenv.axon_hooks" not in sys.modules:
    _hook = [None]
    _m = types.ModuleType("antenv.axon_hooks")
    _m.set_axon_ntff_profile_hook = lambda h: _hook.__setitem__(0, h)
    _m.get_axon_ntff_profile_hook = lambda: _hook[0]
    sys.modules["antenv.axon_hooks"] = _m
    try:
        import antenv
        antenv.axon_hooks = _m
        from trn_agent_boot.trn_boot import _ntff_profile_via_ctypes
        _m.set_axon_ntff_profile_hook(
            _ntff_profile_via_ctypes("/opt/axon/libaxon_pjrt.so"))
    except Exception:
        pass

import numpy as np
import ml_dtypes
from dataclasses import dataclass

import concourse.bacc as bacc
import concourse.bass as bass
import concourse.mybir as mybir
import concourse.tile as tile
from concourse.bass_utils import run_bass_kernel_spmd

P = 128
H = 128
F_IN = 116
FEXT = F_IN + 8          # 124: features + one-hot(type) per batch
B = 2
ROW = 256                # g row: [b0 feats 124 | pad 4 | b1 feats 124 | pad 4]
PROW = 512               # paired stream row: [A_b0 | B_b0 | A_b1 | B_b1]
YW = B * H               # 256 output cols (both batches)

F8 = ml_dtypes.float8_e4m3


@dataclass(frozen=True)
class Cfg:
    n: int = 100000      # nodes
    ncores: int = 8
    tiles: int = 98      # dst tiles per core (128 slots each)
    pairs: int = 9       # chunk pairs per tile (2*128-edge chunks each)

    @property
    def chunks(self):
        return 2 * self.pairs

    @property
    def ndst(self):
        return -(-self.n // self.ncores)

    @property
    def npairs_total(self):
        return self.tiles * self.pairs

    @property
    def nchunks_total(self):
        return self.tiles * self.chunks


CFG = Cfg()

f32 = mybir.dt.float32
f16 = mybir.dt.float16
f8 = mybir.dt.float8e4


def _build_program(cfg: Cfg, has_b1: bool):
    nc = bacc.Bacc("TRN2")
    # pair dim kept explicit: [part, paircol, b-half, A/B, 128] — the
    # DoubleRow matmul requires lhsT APs shaped [K, 2, M]
    strm = nc.dram_tensor("strm", [P, cfg.npairs_total, 2, 2, P], f8,
                          kind="ExternalInput")
    oht = nc.dram_tensor("oht", [P, cfg.nchunks_total, P], f8,
                         kind="ExternalInput")
    dcq = nc.dram_tensor("dcq", [P, cfg.tiles], f32, kind="ExternalInput")
    w1p = nc.dram_tensor("w1p", [P, H], f16, kind="ExternalInput")
    if has_b1:
        disc = nc.dram_tensor("disc", [P, cfg.tiles], f32, kind="ExternalInput")
        cct = nc.dram_tensor("cct", [P, cfg.tiles], f32, kind="ExternalInput")
        b1b = nc.dram_tensor("b1b", [P, YW], f32, kind="ExternalInput")
    accd = nc.dram_tensor("acc", [P, YW], f32, kind="ExternalOutput")

    hc = cfg.pairs  # oh chunks on the scalar rail (first half); rest on sync

    with tile.TileContext(nc) as tc:
        with (
            tc.tile_pool(name="const", bufs=1) as cpool,
            tc.tile_pool(name="gt", bufs=4) as gtpool,
            tc.tile_pool(name="oh", bufs=4) as ohpool,
            tc.tile_pool(name="a1", bufs=6) as a1pool,
            tc.tile_pool(name="x1c", bufs=4) as xpool,
            tc.tile_pool(name="psa", bufs=3, space="PSUM") as psa,
            tc.tile_pool(name="psz", bufs=3, space="PSUM") as psz,
        ):
            w1_sb = cpool.tile([P, H], f16, tag="w1")
            nc.sync.dma_start(w1_sb[:], w1p[:])
            dcq_sb = cpool.tile([P, cfg.tiles], f32, tag="dcq")
            nc.sync.dma_start(dcq_sb[:], dcq[:])
            if has_b1:
                disc_sb = cpool.tile([P, cfg.tiles], f32, tag="disc")
                nc.sync.dma_start(disc_sb[:], disc[:])
                cc_sb = cpool.tile([P, cfg.tiles], f32, tag="cc")
                nc.sync.dma_start(cc_sb[:], cct[:])
                b1_sb = cpool.tile([P, YW], f32, tag="b1b")
                nc.sync.dma_start(b1_sb[:], b1b[:])
            acc_sb = cpool.tile([P, YW], f32, tag="acc")
            nc.vector.memset(acc_sb[:], 0)

            pending = None   # (tile idx, a01 tile) awaiting its z/relu/acc
            last_z1 = None   # previous tile's final z matmul (PE ordering)

            for t in range(cfg.tiles):
                p0 = t * cfg.pairs
                c0 = t * cfg.chunks
                gt = gtpool.tile([P, cfg.pairs, 2, 2, P], f8, tag="gt")
                nc.gpsimd.dma_start(gt[:], strm[:, p0:p0 + cfg.pairs])
                oh = ohpool.tile([P, cfg.chunks, P], f8, tag="oh")
                nc.gpsimd.dma_start(oh[:, 0:hc, :], oht[:, c0:c0 + hc, :])
                nc.sync.dma_start(oh[:, hc:cfg.chunks, :],
                                  oht[:, c0 + hc:c0 + cfg.chunks, :])

                # feature-major segment sum: ps[f, slot], fp8 DoubleRow
                # (each matmul contracts a pair of 128-edge chunks)
                ps = psa.tile([P, 512], f32, tag="psa")
                start_mm = None
                stop_mm = None
                for jp in range(cfg.pairs):
                    for h in range(2):
                        mm = nc.tensor.matmul(
                            ps[:, h * P:(h + 1) * P],
                            lhsT=gt[:, jp, h],
                            rhs=oh[:, 2 * jp:2 * jp + 2, :],
                            start=(jp == 0 and h == 0),
                            stop=(jp == cfg.pairs - 1 and h == 1),
                            perf_mode=mybir.MatmulPerfMode.DoubleRow)
                        if start_mm is None:
                            start_mm = mm
                            if last_z1 is not None:
                                # keep PE order: ... z(t-2), agg(t), z(t-1)
                                bass._add_dep_helper(
                                    mm.ins, last_z1.ins, sync=False,
                                    reason="agg after prev deferred z")
                        else:
                            bass._add_dep_helper(
                                mm.ins, start_mm.ins, sync=False,
                                reason="accum after psum start")
                        stop_mm = mm

                a01 = a1pool.tile([P, YW], f16, tag="a01")
                nc.vector.tensor_copy(a01[:], ps[:, 0:YW])

                def tail(tp, a01p, after_mm):
                    nonlocal last_z1
                    pz = psz.tile([P, 512], f32, tag="psz",
                                  name=f"psz_{tp}")
                    z0 = nc.tensor.matmul(pz[:, 0:P], lhsT=a01p[:, 0:P],
                                          rhs=w1_sb[:], start=True,
                                          stop=False)
                    if after_mm is not None:
                        # deferred z runs right after this tile's agg stop,
                        # never inside an accumulation group and never
                        # waiting on its own a01 CAST (finished a tile ago)
                        bass._add_dep_helper(
                            z0.ins, after_mm.ins, sync=False,
                            reason="deferred z after current agg")
                    z1 = nc.tensor.matmul(pz[:, P:2 * P],
                                          lhsT=a01p[:, P:2 * P],
                                          rhs=w1_sb[:], start=False,
                                          stop=True)
                    bass._add_dep_helper(z1.ins, z0.ins, sync=False,
                                         reason="z1 after psum start")
                    last_z1 = z1
                    x1c = xpool.tile([P, YW], f32, tag="x1c",
                                     name=f"x1c_{tp}")
                    if not has_b1:
                        # x1c = relu(z * (dis*c))   (valid since c>0)
                        nc.scalar.activation(
                            out=x1c[:], in_=pz[:, 0:YW],
                            func=mybir.ActivationFunctionType.Relu,
                            bias=0.0, scale=dcq_sb[:, tp:tp + 1])
                    else:
                        t1 = xpool.tile([P, YW], f32, tag="t1",
                                        name=f"t1_{tp}")
                        nc.vector.tensor_scalar(
                            out=t1[:], in0=pz[:, 0:YW],
                            scalar1=disc_sb[:, tp:tp + 1], scalar2=None,
                            op0=mybir.AluOpType.mult)
                        nc.vector.tensor_tensor(
                            out=t1[:], in0=t1[:], in1=b1_sb[:],
                            op=mybir.AluOpType.add)
                        nc.scalar.activation(
                            out=t1[:], in_=t1[:],
                            func=mybir.ActivationFunctionType.Relu)
                        nc.vector.tensor_scalar(
                            out=x1c[:], in0=t1[:],
                            scalar1=cc_sb[:, tp:tp + 1], scalar2=None,
                            op0=mybir.AluOpType.mult)
                    nc.vector.tensor_tensor(
                        out=acc_sb[:], in0=acc_sb[:], in1=x1c[:],
                        op=mybir.AluOpType.add)

                if pending is not None:
                    tail(pending[0], pending[1], stop_mm)
                pending = (t, a01)

            tail(pending[0], pending[1], None)
            nc.sync.dma_start(accd[:], acc_sb[:])

    nc.compile()
    return nc


_PROG_CACHE = {}


def _get_program(cfg: Cfg, has_b1: bool):
    key = (cfg, has_b1)
    if key not in _PROG_CACHE:
        _PROG_CACHE[key] = _build_program(cfg, has_b1)
    return _PROG_CACHE[key]


def _pack_core(cfg: Cfg, cnt):
    """Bin-pack local dst nodes (cnt = edges incl. self per node) into tiles
    of <=128 slots and <=chunks*128 edges.  Returns (tile_of, slot_of) or
    None if infeasible."""
    ndst = len(cnt)
    cap_e = np.full(cfg.tiles, cfg.chunks * P, dtype=np.int64)
    cap_s = np.full(cfg.tiles, P, dtype=np.int64)
    tile_of = np.empty(ndst, dtype=np.int64)
    slot_of = np.empty(ndst, dtype=np.int64)
    order = np.argsort(-cnt, kind="stable")
    for nloc in order:
        need = cnt[nloc]
        score = np.where((cap_e >= need) & (cap_s > 0), cap_e, -1)
        t = int(np.argmax(score))
        if score[t] < 0:
            return None
        tile_of[nloc] = t
        slot_of[nloc] = P - cap_s[t]
        cap_s[t] -= 1
        cap_e[t] -= need
    return tile_of, slot_of


def _prepare(cfg: Cfg, node, node_type, edge_index, embed, W1, b1, W2, b2):
    n = cfg.n
    src = edge_index[0].astype(np.int64)
    dst = edge_index[1].astype(np.int64)
    deg = (np.bincount(dst, minlength=n) + 1).astype(np.float64)
    dis = 1.0 / np.sqrt(deg)
    s_arr = np.bincount(src, weights=dis[dst], minlength=n)
    c = dis * (s_arr + dis)
    dis_c = (dis * c).astype(np.float32)
    dis32 = dis.astype(np.float32)

    # message-row table: g[n] = dis[n] * [x_b0 | onehot | pad | x_b1 | onehot | pad]
    xg = np.zeros((n, ROW), dtype=F8)
    for b in range(B):
        o = b * 128
        xg[:, o:o + F_IN] = (node[b] * dis32[:, None]).astype(F8)
        oh = np.zeros((n, 8), dtype=np.float32)
        oh[np.arange(n), node_type.astype(np.int64)] = dis32
        xg[:, o + F_IN:o + FEXT] = oh.astype(F8)

    T8 = embed.astype(np.float64) @ W1[F_IN:, :].astype(np.float64)
    w1p = np.zeros((P, H), dtype=np.float16)
    w1p[:F_IN] = W1[:F_IN].astype(np.float16)
    w1p[F_IN:FEXT] = T8.astype(np.float16)

    eye8 = np.eye(P, dtype=F8)

    has_b1 = bool(np.any(b1 != 0))
    in_maps = []
    for core in range(cfg.ncores):
        n0 = core * cfg.ndst
        n1 = min(n0 + cfg.ndst, n)
        sel = (dst >= n0) & (dst < n1)
        es = src[sel]
        edl = dst[sel] - n0
        # append self edges
        own = np.arange(n0, n1, dtype=np.int64)
        es = np.concatenate([es, own])
        edl = np.concatenate([edl, own - n0])

        cnt = np.bincount(edl, minlength=n1 - n0)
        pack = _pack_core(cfg, cnt)
        if pack is None:
            raise RuntimeError(f"core {core}: bin packing failed "
                               f"(tiles={cfg.tiles}, chunks={cfg.chunks})")
        tile_of, slot_of = pack

        et = tile_of[edl]
        order = np.argsort(et, kind="stable")
        et_s = et[order]
        src_s = es[order]
        slot_s = slot_of[edl][order]
        starts = np.concatenate(
            [[0], np.cumsum(np.bincount(et_s, minlength=cfg.tiles))[:-1]])
        rank = np.arange(len(et_s)) - starts[et_s]
        chunk = rank // P
        lane = rank % P
        jp = chunk // 2        # pair index within tile
        ab = chunk % 2         # 0 = A (first of pair), 1 = B
        pc = et_s * cfg.pairs + jp

        strm = np.zeros((P, cfg.npairs_total, PROW), dtype=F8)
        rows = xg[src_s]                               # [E, 256]
        colbase = (ab * P)[:, None] + np.arange(P)[None, :]   # [E, 128]
        strm[lane[:, None], pc[:, None], colbase] = rows[:, 0:P]
        strm[lane[:, None], pc[:, None], 2 * P + colbase] = rows[:, P:2 * P]

        oht = np.zeros((P, cfg.nchunks_total, P), dtype=F8)
        oht[lane, et_s * cfg.chunks + chunk] = eye8[slot_s]

        dcq_w = np.zeros((P, cfg.tiles), dtype=np.float32)
        dcq_w[slot_of, tile_of] = dis_c[n0:n1]

        m = {"strm": strm.reshape(P, cfg.npairs_total, 2, 2, P),
             "oht": oht, "dcq": dcq_w, "w1p": w1p}
        if has_b1:
            disc_w = np.zeros((P, cfg.tiles), dtype=np.float32)
            cc_w = np.zeros((P, cfg.tiles), dtype=np.float32)
            disc_w[slot_of, tile_of] = dis32[n0:n1]
            cc_w[slot_of, tile_of] = c[n0:n1].astype(np.float32)
            m["disc"] = disc_w
            m["cct"] = cc_w
            m["b1b"] = np.tile(b1.astype(np.float32), (P, B))
        in_maps.append(m)
    return in_maps, has_b1


def run(inputs, cfg: Cfg = CFG, trace: bool = False, trace_cores=None):
    node = np.asarray(inputs["node"], dtype=np.float32)
    node_type = np.asarray(inputs["node_type"])
    edge_index = np.asarray(inputs["edge_index"])
    embed = np.asarray(inputs["embed"], dtype=np.float32)
    W1 = np.asarray(inputs["W1"], dtype=np.float32)
    b1 = np.asarray(inputs["b1"], dtype=np.float32)
    W2 = np.asarray(inputs["W2"], dtype=np.float32)
    b2 = np.asarray(inputs["b2"], dtype=np.float32)

    while True:
        try:
            in_maps, has_b1 = _prepare(cfg, node, node_type, edge_index,
                                       embed, W1, b1, W2, b2)
            break
        except RuntimeError:
            # packing infeasible for this edge distribution: add capacity
            cfg = Cfg(n=cfg.n, ncores=cfg.ncores, tiles=cfg.tiles + 2,
                      pairs=cfg.pairs)
    nc = _get_program(cfg, has_b1)
    if trace_cores is None:
        trace_cores = list(range(cfg.ncores))
    res = run_bass_kernel_spmd(
        nc, in_maps, core_ids=list(range(cfg.ncores)), trace=trace,
        trace_cores=trace_cores if trace else None)

    total = np.zeros((B, H), dtype=np.float64)
    for core in range(cfg.ncores):
        acc = res.results[core]["acc"].astype(np.float64)   # [128, 2*H]
        total += acc.reshape(P, B, H).sum(axis=0)
    out = (total @ W2.astype(np.float64)) / cfg.n + b2.astype(np.float64)
    return out.astype(np.float32), res


def kernel(**inputs) -> np.ndarray:
    out, _ = run(inputs, CFG, trace=False)
    return out
